# revision 4
# baseline (speedup 1.0000x reference)
"""Multi-head attention (B=4, N=2048, DIM=64, H=8) on 8 TRN2 NeuronCores.

Sharding: head-parallel tensor parallelism. Each core owns one head h:
  - gets full x, plus the head's slices Wq/Wk/Wv (columns of Wqkv) and
    Wproj rows (augmented with a bias row, only on core 0).
  - computes qT/kT (head-dim on partitions), scores transposed S^T = k @ q^T
    so softmax normalization arrives via an appended ones-column on V
    (row DIM of the AV output accumulates sum_m exp(s)).
  - exp() is fused into the mandatory PSUM->SBUF evacuation on ScalarE
    (max-subtraction is skipped: scores are O(1), mathematically exact).
  - proj uses the *unnormalized* AV output with the l-row included so the
    bias row of the augmented Wproj is scaled by l; one tensor_scalar
    multiply by 1/l per output tile then yields proj(out)/l + bias.
  - per-core partial projections are summed on the host (all-reduce).
"""

import os
import sys

import numpy as np

for _p in ("/opt/trn_rl_repo",):
    if os.path.isdir(_p) and _p not in sys.path:
        sys.path.insert(0, _p)

from contextlib import ExitStack

import concourse.bass as bass
import concourse.tile as tile
from concourse import bacc, mybir
from concourse.bass import ds, ts
from concourse.bass_utils import run_bass_kernel_spmd
from concourse.masks import make_identity

B, N, C, H = 4, 2048, 64, 8
SCALE = C ** -0.5
NCORES = 8
P = 128            # SBUF/PSUM partitions
NB = N // P        # 16 token blocks per batch
CH = 1024          # attention column chunk (PSUM tile free size)
NCH = N // CH      # 2
MMF = 512          # max fp32 moving free dim per matmul
F32 = mybir.dt.float32
EXP = mybir.ActivationFunctionType.Exp


def _attn_kernel(ctx, tc, y, x, wq, wk, wv, wp, lscr):
    nc = tc.nc

    consts = ctx.enter_context(tc.tile_pool(name="consts", bufs=1))
    xin = ctx.enter_context(tc.tile_pool(name="xin", bufs=2))
    xTp = ctx.enter_context(tc.tile_pool(name="xTp", bufs=2))
    qTp = ctx.enter_context(tc.tile_pool(name="qTp", bufs=2))
    kTp = ctx.enter_context(tc.tile_pool(name="kTp", bufs=2))
    vp = ctx.enter_context(tc.tile_pool(name="vp", bufs=2))
    pTp = ctx.enter_context(tc.tile_pool(name="pTp", bufs=4))
    oTp = ctx.enter_context(tc.tile_pool(name="oTp", bufs=2))
    lp = ctx.enter_context(tc.tile_pool(name="lp", bufs=2))
    rlp = ctx.enter_context(tc.tile_pool(name="rlp", bufs=2))
    yp = ctx.enter_context(tc.tile_pool(name="yp", bufs=2))

    ps_s = ctx.enter_context(tc.tile_pool(name="ps_s", bufs=2, space="PSUM"))
    ps_av = ctx.enter_context(tc.tile_pool(name="ps_av", bufs=1, space="PSUM"))
    ps_m = ctx.enter_context(tc.tile_pool(name="ps_m", bufs=2, space="PSUM"))

    identity = consts.tile([P, P], F32)
    make_identity(nc, identity)
    wq_sb = consts.tile([C, C], F32)
    nc.sync.dma_start(out=wq_sb, in_=wq)
    wk_sb = consts.tile([C, C], F32)
    nc.sync.dma_start(out=wk_sb, in_=wk)
    wv_sb = consts.tile([C, C], F32)
    nc.sync.dma_start(out=wv_sb, in_=wv)
    wp_sb = consts.tile([C + 1, C], F32)
    nc.sync.dma_start(out=wp_sb, in_=wp)

    for b in range(B):
        # ---- load x[b] and transpose to xT [C, N] (feature dim on partitions)
        xb = xin.tile([P, NB, C], F32, tag="xb")
        nc.sync.dma_start(out=xb, in_=x[b].rearrange("(t p) c -> p t c", p=P))
        xT = xTp.tile([C, N], F32, tag="xT")
        for t in range(NB):
            pst = ps_m.tile([C, P], F32, tag="m")
            nc.tensor.transpose(pst, xb[:, t, :], identity)
            nc.vector.tensor_copy(out=xT[:, ts(t, P)], in_=pst)

        # ---- qT [C, N], duplicated into partitions C..2C for row-packed S
        qT = qTp.tile([P, N], F32, tag="qT")
        for j in range(N // MMF):
            psq = ps_m.tile([C, MMF], F32, tag="m")
            nc.tensor.matmul(psq, lhsT=wq_sb, rhs=xT[:, ts(j, MMF)],
                             start=True, stop=True)
            nc.vector.tensor_copy(out=qT[0:C, ts(j, MMF)], in_=psq)
        nc.sync.dma_start(out=qT[C:P, :], in_=qT[0:C, :])

        # ---- kT: m-tile 2i at partitions [0:C], m-tile 2i+1 at [C:P]
        kT = kTp.tile([P, NB // 2, P], F32, tag="kT")
        for j in range(N // MMF):
            psk = ps_m.tile([C, MMF], F32, tag="m")
            nc.tensor.matmul(psk, lhsT=wk_sb, rhs=xT[:, ts(j, MMF)],
                             start=True, stop=True)
            psk4 = psk.rearrange("p (g e m) -> p g e m", g=2, e=2)
            nc.vector.tensor_copy(out=kT[0:C, 2 * j:2 * j + 2, :], in_=psk4[:, :, 0, :])
            nc.vector.tensor_copy(out=kT[C:P, 2 * j:2 * j + 2, :], in_=psk4[:, :, 1, :])

        # ---- v_aug [P, NB, C+1]: natural layout + ones column (row sums)
        vaug = vp.tile([P, NB, C + 1], F32, tag="vaug")
        nc.vector.memset(vaug[:, :, C:C + 1], 1.0)
        for t in range(NB):
            psv = ps_m.tile([P, C], F32, tag="m")
            nc.tensor.matmul(psv, lhsT=xT[:, ts(t, P)], rhs=wv_sb,
                             start=True, stop=True)
            nc.vector.tensor_copy(out=vaug[:, t, 0:C], in_=psv)

        # ---- attention + proj, per column chunk
        rl = rlp.tile([P, NB], F32, tag="rl")
        y_sb = yp.tile([P, NB, C], F32, tag="ysb")
        npairs = NB // 2
        for ch in range(NCH):
            av = ps_av.tile([C + 1, CH], F32, tag="av")
            for i in range(npairs):
                sA = ps_s.tile([P, CH], F32, tag="s")
                sB = ps_s.tile([P, CH], F32, tag="s")
                for s in range(CH // MMF):
                    nc.tensor.matmul(sA[:, ts(s, MMF)], lhsT=kT[0:C, i, :],
                                     rhs=qT[0:C, ds(ch * CH + s * MMF, MMF)],
                                     start=True, stop=True)
                for s in range(CH // MMF):
                    nc.tensor.matmul(sB[:, ts(s, MMF)], lhsT=kT[C:P, i, :],
                                     rhs=qT[C:P, ds(ch * CH + s * MMF, MMF)],
                                     start=True, stop=True)
                pA = pTp.tile([P, CH], F32, tag="p")
                pB = pTp.tile([P, CH], F32, tag="p")
                nc.scalar.activation(pA, sA, EXP, scale=SCALE)
                nc.scalar.activation(pB, sB, EXP, scale=SCALE)
                for s in range(CH // MMF):
                    nc.tensor.matmul(av[:, ts(s, MMF)], lhsT=vaug[:, 2 * i, :],
                                     rhs=pA[:, ts(s, MMF)],
                                     start=(i == 0), stop=False)
                    nc.tensor.matmul(av[:, ts(s, MMF)], lhsT=vaug[:, 2 * i + 1, :],
                                     rhs=pB[:, ts(s, MMF)],
                                     start=False, stop=(i == npairs - 1))

            oT = oTp.tile([C + 1, CH], F32, tag="oT")
            nc.vector.tensor_copy(out=oT, in_=av)

            # 1/l in token-block layout via a DRAM bounce
            nc.sync.dma_start(out=lscr[b, ds(ch * CH, CH)][None, :],
                              in_=oT[C:C + 1, :])
            lsc = lp.tile([P, CH // P], F32, tag="lsc")
            nc.sync.dma_start(
                out=lsc,
                in_=lscr[b, ds(ch * CH, CH)].rearrange("(t p) -> p t", p=P))
            nc.vector.reciprocal(out=rl[:, ds(ch * (CH // P), CH // P)], in_=lsc)

            for tt in range(CH // P):
                t = ch * (CH // P) + tt
                psy = ps_m.tile([P, C], F32, tag="m")
                nc.tensor.matmul(psy, lhsT=oT[:, ts(tt, P)], rhs=wp_sb,
                                 start=True, stop=True)
                nc.vector.tensor_scalar_mul(out=y_sb[:, t, :], in0=psy,
                                            scalar1=rl[:, t:t + 1])

        nc.sync.dma_start(out=y[b].rearrange("(t p) c -> p t c", p=P), in_=y_sb)


def build_kernel_nc():
    nc = bacc.Bacc("TRN2", target_bir_lowering=False, debug=False,
                   num_devices=NCORES)
    x = nc.dram_tensor("x", [B, N, C], F32, kind="ExternalInput").ap()
    wq = nc.dram_tensor("wq", [C, C], F32, kind="ExternalInput").ap()
    wk = nc.dram_tensor("wk", [C, C], F32, kind="ExternalInput").ap()
    wv = nc.dram_tensor("wv", [C, C], F32, kind="ExternalInput").ap()
    wp = nc.dram_tensor("wp", [C + 1, C], F32, kind="ExternalInput").ap()
    y = nc.dram_tensor("y", [B, N, C], F32, kind="ExternalOutput").ap()
    lscr = nc.dram_tensor("lscr", [B, N], F32).ap()
    with tile.TileContext(nc) as tc:
        with ExitStack() as ctx:
            _attn_kernel(ctx, tc, y, x, wq, wk, wv, wp, lscr)
    nc.compile()
    return nc


def make_in_maps(x, Wqkv, Wproj, bproj):
    x = np.ascontiguousarray(x, dtype=np.float32)
    Wqkv = np.asarray(Wqkv, dtype=np.float32)
    Wproj = np.asarray(Wproj, dtype=np.float32)
    bproj = np.asarray(bproj, dtype=np.float32)
    in_maps = []
    for h in range(NCORES):
        wq = np.ascontiguousarray(Wqkv[:, 0 * H * C + h * C:0 * H * C + (h + 1) * C])
        wk = np.ascontiguousarray(Wqkv[:, 1 * H * C + h * C:1 * H * C + (h + 1) * C])
        wv = np.ascontiguousarray(Wqkv[:, 2 * H * C + h * C:2 * H * C + (h + 1) * C])
        brow = bproj if h == 0 else np.zeros_like(bproj)
        wp = np.ascontiguousarray(
            np.concatenate([Wproj[h * C:(h + 1) * C, :], brow[None, :]], axis=0))
        in_maps.append({"x": x, "wq": wq, "wk": wk, "wv": wv, "wp": wp})
    return in_maps


_NC_CACHE = None


def _get_nc():
    global _NC_CACHE
    if _NC_CACHE is None:
        _NC_CACHE = build_kernel_nc()
    return _NC_CACHE


def run(inputs, trace=False, trace_kwargs=None):
    in_maps = make_in_maps(**inputs)
    res = run_bass_kernel_spmd(_get_nc(), in_maps, list(range(NCORES)),
                               trace=trace, **(trace_kwargs or {}))
    y = np.zeros((B, N, C), np.float32)
    for r in res.results:
        y += r["y"]
    return y, res


def kernel(x, Wqkv, Wproj, bproj):
    y, _ = run(dict(x=x, Wqkv=Wqkv, Wproj=Wproj, bproj=bproj))
    return y


# revision 21
# speedup vs baseline: 1.4940x; 1.4940x over previous
"""Multi-head attention (B=4, N=2048, DIM=64, H=8) on 8 TRN2 NeuronCores.

Sharding: head-parallel tensor parallelism. Each core owns one head h:
  - gets full x, plus the head's slices Wq/Wk/Wv (columns of Wqkv) and
    Wproj rows (augmented with a bias row, only on core 0).
  - computes qT/kT (head-dim on partitions), scores transposed S^T = k @ q^T
    so softmax normalization arrives via an appended ones-column on V
    (row DIM of the AV output accumulates sum_m exp(s)).
  - exp() is fused into the mandatory PSUM->SBUF evacuation on ScalarE
    (max-subtraction is skipped: scores are O(1), mathematically exact).
  - proj uses the *unnormalized* AV output with the l-row included so the
    bias row of the augmented Wproj is scaled by l; one tensor_scalar
    multiply by 1/l per output tile then yields proj(out)/l + bias.
  - per-core partial projections are summed on the host (all-reduce).
"""

import os
import sys

import numpy as np

for _p in ("/opt/trn_rl_repo",):
    if os.path.isdir(_p) and _p not in sys.path:
        sys.path.insert(0, _p)

from contextlib import ExitStack

import concourse.bass as bass
import concourse.tile as tile
from concourse import bacc, mybir
from concourse.bass import ds, ts
from concourse.bass_utils import run_bass_kernel_spmd
from concourse.masks import make_identity

B, N, C, H = 4, 2048, 64, 8
SCALE = C ** -0.5
NCORES = 8
P = 128            # SBUF/PSUM partitions
NB = N // P        # 16 token blocks per batch
CH = 1024          # attention column chunk (PSUM tile free size)
NCH = N // CH      # 2
MMF = 512          # max fp32 moving free dim per matmul
F32 = mybir.dt.float32
# dtype for the attention matmuls (scores and attn@V): float32r streams at
# 1 cycle/row on the PE (vs 4 for fp32 two-pass) at ~1.7e-4 relative error.
SDT = mybir.dt.float32r
EXP = mybir.ActivationFunctionType.Exp


def _attn_kernel(ctx, tc, y, x, wq, wk, wv, wp, lscr):
    nc = tc.nc

    consts = ctx.enter_context(tc.tile_pool(name="consts", bufs=1))
    xin = ctx.enter_context(tc.tile_pool(name="xin", bufs=2))
    xTp = ctx.enter_context(tc.tile_pool(name="xTp", bufs=2))
    qTp = ctx.enter_context(tc.tile_pool(name="qTp", bufs=2))
    kTp = ctx.enter_context(tc.tile_pool(name="kTp", bufs=2))
    vp = ctx.enter_context(tc.tile_pool(name="vp", bufs=2))
    pTp = ctx.enter_context(tc.tile_pool(name="pTp", bufs=4))
    oTp = ctx.enter_context(tc.tile_pool(name="oTp", bufs=2))
    lp = ctx.enter_context(tc.tile_pool(name="lp", bufs=2))
    rlp = ctx.enter_context(tc.tile_pool(name="rlp", bufs=2))
    yp = ctx.enter_context(tc.tile_pool(name="yp", bufs=2))

    ps_s = ctx.enter_context(tc.tile_pool(name="ps_s", bufs=2, space="PSUM"))
    ps_av = ctx.enter_context(tc.tile_pool(name="ps_av", bufs=1, space="PSUM"))
    ps_m = ctx.enter_context(tc.tile_pool(name="ps_m", bufs=2, space="PSUM"))

    identity = consts.tile([P, P], F32)
    make_identity(nc, identity)
    wq_sb = consts.tile([C, C], F32)
    nc.sync.dma_start(out=wq_sb, in_=wq)
    wk_sb = consts.tile([C, C], F32)
    nc.sync.dma_start(out=wk_sb, in_=wk)
    wv_sb = consts.tile([C, C], F32)
    nc.sync.dma_start(out=wv_sb, in_=wv)
    wp_sb = consts.tile([C + 1, C], F32)
    nc.sync.dma_start(out=wp_sb, in_=wp)

    for b in range(B):
        # ---- load x[b] and transpose to xT [C, N] (feature dim on partitions)
        xb = xin.tile([P, NB, C], F32, tag="xb")
        nc.sync.dma_start(out=xb, in_=x[b].rearrange("(t p) c -> p t c", p=P))
        xT = xTp.tile([C, N], F32, tag="xT")
        for t in range(NB):
            pst = ps_m.tile([C, P], F32, tag="m")
            nc.tensor.transpose(pst, xb[:, t, :], identity)
            nc.vector.tensor_copy(out=xT[:, ts(t, P)], in_=pst)

        # ---- qT [C, N], duplicated into partitions C..2C for row-packed S
        qT = qTp.tile([P, N], SDT, tag="qT")
        for j in range(N // MMF):
            psq = ps_m.tile([C, MMF], F32, tag="m")
            nc.tensor.matmul(psq, lhsT=wq_sb, rhs=xT[:, ts(j, MMF)],
                             start=True, stop=True)
            nc.vector.tensor_copy(out=qT[0:C, ts(j, MMF)], in_=psq)
        nc.sync.dma_start(out=qT[C:P, :], in_=qT[0:C, :])

        # ---- kT: m-tile 2i at partitions [0:C], m-tile 2i+1 at [C:P]
        kT = kTp.tile([P, NB // 2, P], SDT, tag="kT")
        for j in range(N // MMF):
            psk = ps_m.tile([C, MMF], F32, tag="m")
            nc.tensor.matmul(psk, lhsT=wk_sb, rhs=xT[:, ts(j, MMF)],
                             start=True, stop=True)
            psk4 = psk.rearrange("p (g e m) -> p g e m", g=2, e=2)
            nc.vector.tensor_copy(out=kT[0:C, 2 * j:2 * j + 2, :], in_=psk4[:, :, 0, :])
            nc.vector.tensor_copy(out=kT[C:P, 2 * j:2 * j + 2, :], in_=psk4[:, :, 1, :])

        # ---- v_aug [P, NB, C+1]: natural layout + ones column (row sums)
        vaug = vp.tile([P, NB, C + 1], SDT, tag="vaug")
        ones_f32 = vp.tile([P, NB], F32, tag="ones")
        nc.vector.memset(ones_f32, 1.0)
        nc.vector.tensor_copy(out=vaug[:, :, C], in_=ones_f32)
        for t in range(NB):
            psv = ps_m.tile([P, C], F32, tag="m")
            nc.tensor.matmul(psv, lhsT=xT[:, ts(t, P)], rhs=wv_sb,
                             start=True, stop=True)
            nc.vector.tensor_copy(out=vaug[:, t, 0:C], in_=psv)

        # ---- attention + proj, per column chunk
        rl = rlp.tile([P, NB], F32, tag="rl")
        y_sb = yp.tile([P, NB, C], F32, tag="ysb")
        npairs = NB // 2
        for ch in range(NCH):
            av = ps_av.tile([C + 1, CH], F32, tag="av")
            for i in range(npairs):
                sA = ps_s.tile([P, CH], F32, tag="s")
                sB = ps_s.tile([P, CH], F32, tag="s")
                for s in range(CH // MMF):
                    nc.tensor.matmul(sA[:, ts(s, MMF)], lhsT=kT[0:C, i, :],
                                     rhs=qT[0:C, ds(ch * CH + s * MMF, MMF)],
                                     start=True, stop=True)
                for s in range(CH // MMF):
                    nc.tensor.matmul(sB[:, ts(s, MMF)], lhsT=kT[C:P, i, :],
                                     rhs=qT[C:P, ds(ch * CH + s * MMF, MMF)],
                                     start=True, stop=True)
                pA = pTp.tile([P, CH], SDT, tag="p")
                pB = pTp.tile([P, CH], SDT, tag="p")
                nc.scalar.activation(pA, sA, EXP, scale=SCALE)
                nc.scalar.activation(pB, sB, EXP, scale=SCALE)
                for s in range(CH // MMF):
                    nc.tensor.matmul(av[:, ts(s, MMF)], lhsT=vaug[:, 2 * i, :],
                                     rhs=pA[:, ts(s, MMF)],
                                     start=(i == 0), stop=False)
                    nc.tensor.matmul(av[:, ts(s, MMF)], lhsT=vaug[:, 2 * i + 1, :],
                                     rhs=pB[:, ts(s, MMF)],
                                     start=False, stop=(i == npairs - 1))

            oT = oTp.tile([C + 1, CH], F32, tag="oT")
            nc.vector.tensor_copy(out=oT, in_=av)

            # 1/l in token-block layout via a DRAM bounce
            nc.sync.dma_start(out=lscr[b, ds(ch * CH, CH)][None, :],
                              in_=oT[C:C + 1, :])
            lsc = lp.tile([P, CH // P], F32, tag="lsc")
            nc.sync.dma_start(
                out=lsc,
                in_=lscr[b, ds(ch * CH, CH)].rearrange("(t p) -> p t", p=P))
            nc.vector.reciprocal(out=rl[:, ds(ch * (CH // P), CH // P)], in_=lsc)

            for tt in range(CH // P):
                t = ch * (CH // P) + tt
                psy = ps_m.tile([P, C], F32, tag="m")
                nc.tensor.matmul(psy, lhsT=oT[:, ts(tt, P)], rhs=wp_sb,
                                 start=True, stop=True)
                nc.vector.tensor_scalar_mul(out=y_sb[:, t, :], in0=psy,
                                            scalar1=rl[:, t:t + 1])

        nc.sync.dma_start(out=y[b].rearrange("(t p) c -> p t c", p=P), in_=y_sb)


def build_kernel_nc():
    nc = bacc.Bacc("TRN2", target_bir_lowering=False, debug=False,
                   num_devices=NCORES)
    x = nc.dram_tensor("x", [B, N, C], F32, kind="ExternalInput").ap()
    wq = nc.dram_tensor("wq", [C, C], F32, kind="ExternalInput").ap()
    wk = nc.dram_tensor("wk", [C, C], F32, kind="ExternalInput").ap()
    wv = nc.dram_tensor("wv", [C, C], F32, kind="ExternalInput").ap()
    wp = nc.dram_tensor("wp", [C + 1, C], F32, kind="ExternalInput").ap()
    y = nc.dram_tensor("y", [B, N, C], F32, kind="ExternalOutput").ap()
    lscr = nc.dram_tensor("lscr", [B, N], F32).ap()
    with tile.TileContext(nc) as tc:
        with ExitStack() as ctx:
            _attn_kernel(ctx, tc, y, x, wq, wk, wv, wp, lscr)
    nc.compile()
    return nc


def make_in_maps(x, Wqkv, Wproj, bproj):
    x = np.ascontiguousarray(x, dtype=np.float32)
    Wqkv = np.asarray(Wqkv, dtype=np.float32)
    Wproj = np.asarray(Wproj, dtype=np.float32)
    bproj = np.asarray(bproj, dtype=np.float32)
    in_maps = []
    for h in range(NCORES):
        wq = np.ascontiguousarray(Wqkv[:, 0 * H * C + h * C:0 * H * C + (h + 1) * C])
        wk = np.ascontiguousarray(Wqkv[:, 1 * H * C + h * C:1 * H * C + (h + 1) * C])
        wv = np.ascontiguousarray(Wqkv[:, 2 * H * C + h * C:2 * H * C + (h + 1) * C])
        brow = bproj if h == 0 else np.zeros_like(bproj)
        wp = np.ascontiguousarray(
            np.concatenate([Wproj[h * C:(h + 1) * C, :], brow[None, :]], axis=0))
        in_maps.append({"x": x, "wq": wq, "wk": wk, "wv": wv, "wp": wp})
    return in_maps


_NC_CACHE = None


def _get_nc():
    global _NC_CACHE
    if _NC_CACHE is None:
        _NC_CACHE = build_kernel_nc()
    return _NC_CACHE


def run(inputs, trace=False, trace_kwargs=None):
    in_maps = make_in_maps(**inputs)
    res = run_bass_kernel_spmd(_get_nc(), in_maps, list(range(NCORES)),
                               trace=trace, **(trace_kwargs or {}))
    y = np.zeros((B, N, C), np.float32)
    for r in res.results:
        y += r["y"]
    return y, res


def kernel(x, Wqkv, Wproj, bproj):
    y, _ = run(dict(x=x, Wqkv=Wqkv, Wproj=Wproj, bproj=bproj))
    return y


# revision 23
# speedup vs baseline: 1.6451x; 1.1012x over previous
"""Multi-head attention (B=4, N=2048, DIM=64, H=8) on 8 TRN2 NeuronCores.

Sharding: head-parallel tensor parallelism. Each core owns one head h:
  - gets full x, plus the head's slices Wq/Wk/Wv (columns of Wqkv) and
    Wproj rows (augmented with a bias row, only on core 0).
  - computes qT/kT (head-dim on partitions), scores transposed S^T = k @ q^T
    so softmax normalization arrives via an appended ones-column on V
    (row DIM of the AV output accumulates sum_m exp(s)).
  - exp() is fused into the mandatory PSUM->SBUF evacuation on ScalarE
    (max-subtraction is skipped: scores are O(1), mathematically exact).
  - proj uses the *unnormalized* AV output with the l-row included so the
    bias row of the augmented Wproj is scaled by l; one tensor_scalar
    multiply by 1/l per output tile then yields proj(out)/l + bias.
  - per-core partial projections are summed on the host (all-reduce).
"""

import os
import sys

import numpy as np

for _p in ("/opt/trn_rl_repo",):
    if os.path.isdir(_p) and _p not in sys.path:
        sys.path.insert(0, _p)

from contextlib import ExitStack

import concourse.bass as bass
import concourse.tile as tile
from concourse import bacc, mybir
from concourse.bass import ds, ts
from concourse.bass_utils import run_bass_kernel_spmd
from concourse.masks import make_identity

B, N, C, H = 4, 2048, 64, 8
SCALE = C ** -0.5
NCORES = 8
P = 128            # SBUF/PSUM partitions
NB = N // P        # 16 token blocks per batch
CH = 1024          # attention column chunk (PSUM tile free size)
NCH = N // CH      # 2
MMF = 512          # max fp32 moving free dim per matmul
F32 = mybir.dt.float32
# dtype for the attention matmuls (scores and attn@V). The PE moving-operand
# stream is ~32 bits/partition/1.2GHz-cycle, so 2-byte operands double matmul
# throughput; bf16 also enables FastWeightLoad. float32r is the higher
# precision fallback (~1.7e-4 vs ~4e-3 relative).
SDT = mybir.dt.bfloat16
EXP = mybir.ActivationFunctionType.Exp


def _attn_kernel(ctx, tc, y, x, wq, wk, wv, wp, lscr):
    nc = tc.nc

    consts = ctx.enter_context(tc.tile_pool(name="consts", bufs=1))
    xin = ctx.enter_context(tc.tile_pool(name="xin", bufs=2))
    xTp = ctx.enter_context(tc.tile_pool(name="xTp", bufs=2))
    qTp = ctx.enter_context(tc.tile_pool(name="qTp", bufs=2))
    kTp = ctx.enter_context(tc.tile_pool(name="kTp", bufs=2))
    vp = ctx.enter_context(tc.tile_pool(name="vp", bufs=2))
    pTp = ctx.enter_context(tc.tile_pool(name="pTp", bufs=4))
    oTp = ctx.enter_context(tc.tile_pool(name="oTp", bufs=2))
    lp = ctx.enter_context(tc.tile_pool(name="lp", bufs=2))
    rlp = ctx.enter_context(tc.tile_pool(name="rlp", bufs=2))
    yp = ctx.enter_context(tc.tile_pool(name="yp", bufs=2))

    ps_s = ctx.enter_context(tc.tile_pool(name="ps_s", bufs=2, space="PSUM"))
    ps_av = ctx.enter_context(tc.tile_pool(name="ps_av", bufs=1, space="PSUM"))
    ps_m = ctx.enter_context(tc.tile_pool(name="ps_m", bufs=2, space="PSUM"))

    identity = consts.tile([P, P], F32)
    make_identity(nc, identity)
    wq_sb = consts.tile([C, C], F32)
    nc.sync.dma_start(out=wq_sb, in_=wq)
    wk_sb = consts.tile([C, C], F32)
    nc.sync.dma_start(out=wk_sb, in_=wk)
    wv_sb = consts.tile([C, C], F32)
    nc.sync.dma_start(out=wv_sb, in_=wv)
    wp_sb = consts.tile([C + 1, C], F32)
    nc.sync.dma_start(out=wp_sb, in_=wp)

    for b in range(B):
        # ---- load x[b] and transpose to xT [C, N] (feature dim on partitions)
        xb = xin.tile([P, NB, C], F32, tag="xb")
        nc.sync.dma_start(out=xb, in_=x[b].rearrange("(t p) c -> p t c", p=P))
        xT = xTp.tile([C, N], F32, tag="xT")
        for t in range(NB):
            pst = ps_m.tile([C, P], F32, tag="m")
            nc.tensor.transpose(pst, xb[:, t, :], identity)
            nc.vector.tensor_copy(out=xT[:, ts(t, P)], in_=pst)

        # ---- qT [C, N], duplicated into partitions C..2C for row-packed S
        qT = qTp.tile([P, N], SDT, tag="qT")
        for j in range(N // MMF):
            psq = ps_m.tile([C, MMF], F32, tag="m")
            nc.tensor.matmul(psq, lhsT=wq_sb, rhs=xT[:, ts(j, MMF)],
                             start=True, stop=True)
            nc.vector.tensor_copy(out=qT[0:C, ts(j, MMF)], in_=psq)
        nc.sync.dma_start(out=qT[C:P, :], in_=qT[0:C, :])

        # ---- kT: m-tile 2i at partitions [0:C], m-tile 2i+1 at [C:P]
        kT = kTp.tile([P, NB // 2, P], SDT, tag="kT")
        for j in range(N // MMF):
            psk = ps_m.tile([C, MMF], F32, tag="m")
            nc.tensor.matmul(psk, lhsT=wk_sb, rhs=xT[:, ts(j, MMF)],
                             start=True, stop=True)
            psk4 = psk.rearrange("p (g e m) -> p g e m", g=2, e=2)
            nc.vector.tensor_copy(out=kT[0:C, 2 * j:2 * j + 2, :], in_=psk4[:, :, 0, :])
            nc.vector.tensor_copy(out=kT[C:P, 2 * j:2 * j + 2, :], in_=psk4[:, :, 1, :])

        # ---- v_aug [P, NB, C+1]: natural layout + ones column (row sums)
        vaug = vp.tile([P, NB, C + 1], SDT, tag="vaug")
        ones_f32 = vp.tile([P, NB], F32, tag="ones")
        nc.vector.memset(ones_f32, 1.0)
        nc.vector.tensor_copy(out=vaug[:, :, C], in_=ones_f32)
        for t in range(NB):
            psv = ps_m.tile([P, C], F32, tag="m")
            nc.tensor.matmul(psv, lhsT=xT[:, ts(t, P)], rhs=wv_sb,
                             start=True, stop=True)
            nc.vector.tensor_copy(out=vaug[:, t, 0:C], in_=psv)

        # ---- attention + proj, per column chunk
        rl = rlp.tile([P, NB], F32, tag="rl")
        y_sb = yp.tile([P, NB, C], F32, tag="ysb")
        npairs = NB // 2
        for ch in range(NCH):
            av = ps_av.tile([C + 1, CH], F32, tag="av")
            for i in range(npairs):
                sA = ps_s.tile([P, CH], F32, tag="s")
                sB = ps_s.tile([P, CH], F32, tag="s")
                # interleave the two row-group members so their streams overlap
                for s in range(CH // MMF):
                    nc.tensor.matmul(sA[:, ts(s, MMF)], lhsT=kT[0:C, i, :],
                                     rhs=qT[0:C, ds(ch * CH + s * MMF, MMF)],
                                     start=True, stop=True)
                    nc.tensor.matmul(sB[:, ts(s, MMF)], lhsT=kT[C:P, i, :],
                                     rhs=qT[C:P, ds(ch * CH + s * MMF, MMF)],
                                     start=True, stop=True)
                pA = pTp.tile([P, CH], SDT, tag="p")
                pB = pTp.tile([P, CH], SDT, tag="p")
                nc.scalar.activation(pA, sA, EXP, scale=SCALE)
                nc.scalar.activation(pB, sB, EXP, scale=SCALE)
                for s in range(CH // MMF):
                    nc.tensor.matmul(av[:, ts(s, MMF)], lhsT=vaug[:, 2 * i, :],
                                     rhs=pA[:, ts(s, MMF)],
                                     start=(i == 0), stop=False)
                    nc.tensor.matmul(av[:, ts(s, MMF)], lhsT=vaug[:, 2 * i + 1, :],
                                     rhs=pB[:, ts(s, MMF)],
                                     start=False, stop=(i == npairs - 1))

            oT = oTp.tile([C + 1, CH], F32, tag="oT")
            nc.vector.tensor_copy(out=oT, in_=av)

            # 1/l in token-block layout via a DRAM bounce
            nc.sync.dma_start(out=lscr[b, ds(ch * CH, CH)][None, :],
                              in_=oT[C:C + 1, :])
            lsc = lp.tile([P, CH // P], F32, tag="lsc")
            nc.sync.dma_start(
                out=lsc,
                in_=lscr[b, ds(ch * CH, CH)].rearrange("(t p) -> p t", p=P))
            nc.vector.reciprocal(out=rl[:, ds(ch * (CH // P), CH // P)], in_=lsc)

            for tt in range(CH // P):
                t = ch * (CH // P) + tt
                psy = ps_m.tile([P, C], F32, tag="m")
                nc.tensor.matmul(psy, lhsT=oT[:, ts(tt, P)], rhs=wp_sb,
                                 start=True, stop=True)
                nc.vector.tensor_scalar_mul(out=y_sb[:, t, :], in0=psy,
                                            scalar1=rl[:, t:t + 1])

        nc.sync.dma_start(out=y[b].rearrange("(t p) c -> p t c", p=P), in_=y_sb)


def build_kernel_nc():
    nc = bacc.Bacc("TRN2", target_bir_lowering=False, debug=False,
                   num_devices=NCORES)
    x = nc.dram_tensor("x", [B, N, C], F32, kind="ExternalInput").ap()
    wq = nc.dram_tensor("wq", [C, C], F32, kind="ExternalInput").ap()
    wk = nc.dram_tensor("wk", [C, C], F32, kind="ExternalInput").ap()
    wv = nc.dram_tensor("wv", [C, C], F32, kind="ExternalInput").ap()
    wp = nc.dram_tensor("wp", [C + 1, C], F32, kind="ExternalInput").ap()
    y = nc.dram_tensor("y", [B, N, C], F32, kind="ExternalOutput").ap()
    lscr = nc.dram_tensor("lscr", [B, N], F32).ap()
    with tile.TileContext(nc) as tc:
        with ExitStack() as ctx:
            _attn_kernel(ctx, tc, y, x, wq, wk, wv, wp, lscr)
    nc.compile()
    return nc


def make_in_maps(x, Wqkv, Wproj, bproj):
    x = np.ascontiguousarray(x, dtype=np.float32)
    Wqkv = np.asarray(Wqkv, dtype=np.float32)
    Wproj = np.asarray(Wproj, dtype=np.float32)
    bproj = np.asarray(bproj, dtype=np.float32)
    in_maps = []
    for h in range(NCORES):
        wq = np.ascontiguousarray(Wqkv[:, 0 * H * C + h * C:0 * H * C + (h + 1) * C])
        wk = np.ascontiguousarray(Wqkv[:, 1 * H * C + h * C:1 * H * C + (h + 1) * C])
        wv = np.ascontiguousarray(Wqkv[:, 2 * H * C + h * C:2 * H * C + (h + 1) * C])
        brow = bproj if h == 0 else np.zeros_like(bproj)
        wp = np.ascontiguousarray(
            np.concatenate([Wproj[h * C:(h + 1) * C, :], brow[None, :]], axis=0))
        in_maps.append({"x": x, "wq": wq, "wk": wk, "wv": wv, "wp": wp})
    return in_maps


_NC_CACHE = None


def _get_nc():
    global _NC_CACHE
    if _NC_CACHE is None:
        _NC_CACHE = build_kernel_nc()
    return _NC_CACHE


def run(inputs, trace=False, trace_kwargs=None):
    in_maps = make_in_maps(**inputs)
    res = run_bass_kernel_spmd(_get_nc(), in_maps, list(range(NCORES)),
                               trace=trace, **(trace_kwargs or {}))
    y = np.zeros((B, N, C), np.float32)
    for r in res.results:
        y += r["y"]
    return y, res


def kernel(x, Wqkv, Wproj, bproj):
    y, _ = run(dict(x=x, Wqkv=Wqkv, Wproj=Wproj, bproj=bproj))
    return y
